# revision 18
# baseline (speedup 1.0000x reference)
"""kernel_b + block-wide bf16 one-hot + quad logits tiles.

TransformerConv + ELU on 8 NeuronCores — baseline compute pipeline with
batched dma_gather from a compacted k|v table.

vs the 1.52 ms baseline, the only structural changes are:
  - the k|v table holds only the ~63k sources referenced by this core
    (host-compacted, capacity 65536), written by phase 1a from a compact
    xTc input: 34 MB less HBM write + 9 MB less read per core.
  - per-edge k|v rows are gathered with ONE nc.gpsimd.dma_gather per
    (2-block supergroup, table half) instead of 794 per-chunk indirect
    DMAs: SWDGE descriptor generation drops from ~824 us to ~140 us and
    the dynamic-DMA queues stop being the critical path. dma_gather takes
    int16 indices, so the table is addressed as two 32768-row halves; a 2D
    LPT packs dsts into blocks against per-half chunk profiles (chA, chB)
    shared by all cores, so each block's half split is compile-time static.
  - pass A of block b+1 is emitted before pass B of block b so the PE
    stream never stalls on the DVE logit chain.
All per-chunk compute (one-hot, PE transpose, Qg, prod/reduce logits, ACT
mex, PE scatter, ELU epilogue) is byte-identical to the baseline.
"""
import math
import numpy as np
import ml_dtypes

BF16 = ml_dtypes.bfloat16

N, E, D = 100000, 800000, 128
M_CORES = 8
DPC = N // M_CORES                 # 12500
NB = (DPC + 127) // 128            # 98
DST_PAD = NB * 128                 # 12544
U_PAD = 65536                      # compact source-table capacity
HALF = 32768                       # int16 index reach per table half
NGRP = NB // 2                     # gather supergroups (2 blocks each)
SCALE = 1.0 / math.sqrt(D)
TW = 4096                          # phase-1a row-tile width
TWB = 2048                         # phase-1b row-tile width


class Prep:
    pass


def _host_prep(edge_index):
    src = np.asarray(edge_index[0], dtype=np.int64)
    dst = np.asarray(edge_index[1], dtype=np.int64)
    core = dst // DPC
    ld = dst - core * DPC

    uniqs, kidxs, e_lds = [], [], []
    degA = np.zeros((M_CORES, DST_PAD), np.int64)
    degB = np.zeros((M_CORES, DST_PAD), np.int64)
    for c in range(M_CORES):
        sel = core == c
        e_src = src[sel]
        e_ld = ld[sel]
        uniq = np.unique(e_src)
        if len(uniq) > U_PAD:
            raise RuntimeError("compact table overflow")
        kidx = np.searchsorted(uniq, e_src).astype(np.int64)
        half = kidx >= HALF
        degA[c, :DPC] = np.bincount(e_ld[~half], minlength=DPC)[:DPC]
        degB[c, :DPC] = np.bincount(e_ld[half], minlength=DPC)[:DPC]
        uniqs.append(uniq)
        kidxs.append(kidx)
        e_lds.append(e_ld)

    maxA = int(degA.sum(axis=1).max())
    maxB = int(degB.sum(axis=1).max())
    slack = 1024
    mA = int(max(0, math.ceil((maxA + slack - 4 * NB * 128) / 128.0)))
    mB = int(max(0, math.ceil((maxB + slack - 4 * NB * 128) / 128.0)))
    while True:
        if mA > NB or mB > NB:
            raise RuntimeError("profile overflow")
        chA = np.array([5] * mA + [4] * (NB - mA), np.int64)
        chB = np.array([5] * mB + [4] * (NB - mB), np.int64)
        capsA = chA * 128
        capsB = chB * 128
        assign = np.zeros((M_CORES, DST_PAD), np.int64)
        okA = okB = True
        for c in range(M_CORES):
            dA, dB = degA[c], degB[c]
            order = np.argsort(-(dA + dB))
            loadsA = np.zeros(NB, np.int64)
            loadsB = np.zeros(NB, np.int64)
            Gk = 14
            for k in range(128):
                batch = order[k * NB:(k + 1) * NB]
                sA = capsA - loadsA
                sB = capsB - loadsB
                batch2 = batch[np.argsort(-(dA[batch] + dB[batch]))]
                bins2 = np.argsort(-(sA + sB))
                for g0 in range(0, NB, Gk):
                    ii = batch2[g0:g0 + Gk]
                    bb = bins2[g0:g0 + Gk]
                    batch2[g0:g0 + Gk] = ii[np.argsort(dA[ii] - dB[ii])]
                    bins2[g0:g0 + Gk] = bb[np.argsort(sA[bb] - sB[bb])]
                assign[c, batch2] = bins2
                loadsA[bins2] += dA[batch2]
                loadsB[bins2] += dB[batch2]
            # repair: swap a heavy dst from the worst overfull bin with the
            # lightest dst of the best-slack bin (keeps 128 dsts per bin)
            for _ in range(8000):
                overA = loadsA - capsA
                overB = loadsB - capsB
                wb = int(np.argmax(np.maximum(overA, overB)))
                if max(overA[wb], overB[wb]) <= 0:
                    break
                offA = overA[wb] >= overB[wb]
                dH, dO = (dA, dB) if offA else (dB, dA)
                members = np.where(assign[c] == wb)[0]
                d = int(members[int(np.argmax(dH[members]))])
                fit = np.maximum(loadsA + dA[d] - capsA,
                                 loadsB + dB[d] - capsB)
                fit[wb] = 1 << 30
                u = int(np.argmin(fit))
                mu = np.where(assign[c] == u)[0]
                e = int(mu[int(np.argmin(dH[mu] * 256 + dO[mu]))])
                ok_move = (fit[u] - dA[e] - dB[e] <= 0 and dH[e] < dH[d]
                           and (loadsB if offA else loadsA)[wb]
                           - dO[d] + dO[e] <= (capsB if offA else capsA)[wb])
                if not ok_move:
                    okA = okB = False
                    break
                assign[c, d] = u
                assign[c, e] = wb
                loadsA[wb] += dA[e] - dA[d]
                loadsB[wb] += dB[e] - dB[d]
                loadsA[u] += dA[d] - dA[e]
                loadsB[u] += dB[d] - dB[e]
            okA &= bool((loadsA <= capsA).all())
            okB &= bool((loadsB <= capsB).all())
            if not (okA and okB):
                break
        if okA and okB:
            break
        if not okA:
            mA += 2
        if not okB:
            mB += 2

    cc = chA + chB
    S = int(cc.sum())
    colbase = np.concatenate([[0], np.cumsum(cc)[:-1]])
    chH = [chA, chB]
    call_ni = np.zeros(NB, np.int64)
    for g in range(NGRP):
        for h in (0, 1):
            call_ni[g * 2 + h] = 128 * (chH[h][2 * g] + chH[h][2 * g + 1])
    call_slotbase = np.concatenate([[0], np.cumsum(call_ni)[:-1]])
    tot_slots = int(call_ni.sum())       # == 128 * S

    idx16 = np.zeros((M_CORES, 128, tot_slots // 16), np.int16)
    dstloc = np.full((M_CORES, 128, S), 255.0, np.float32)
    perm = np.zeros((M_CORES, DST_PAD), np.int64)
    for c in range(M_CORES):
        blk = assign[c]
        order = np.argsort(blk, kind="stable")
        blk_sorted = blk[order]
        starts = np.searchsorted(blk_sorted, np.arange(NB))
        lane = np.arange(DST_PAD) - starts[blk_sorted]
        rows = blk_sorted * 128 + lane
        perm[c, rows] = order
        lane_of = np.zeros(DST_PAD, np.int64)
        lane_of[order] = lane

        e_ld = e_lds[c]
        kidx = kidxs[c]
        e_blk = blk[e_ld]
        flat_idx = np.zeros(tot_slots, np.int64)
        for h in (0, 1):
            sel_h = (kidx >= HALF) == bool(h)
            gb_all = e_blk[sel_h]
            ki_h = kidx[sel_h] - h * HALF
            ld_h = e_ld[sel_h]
            g_order = np.argsort(gb_all, kind="stable")
            gb = gb_all[g_order]
            caps = chH[h] * 128
            counts = np.bincount(gb, minlength=NB)
            if (counts > caps).any():
                raise RuntimeError("packing overflow")
            estarts = np.concatenate([[0], np.cumsum(counts)[:-1]])
            j = np.arange(len(gb)) - estarts[gb]
            lc = j // 128
            lane_e = j % 128
            scol = colbase[gb] + (chA[gb] if h else 0) + lc
            dstloc[c, lane_e, scol] = lane_of[ld_h[g_order]].astype(np.float32)
            k_call = (gb // 2) * 2 + h
            pos = lc + np.where(gb % 2 == 1, chH[h][(gb // 2) * 2], 0)
            slot = call_slotbase[k_call] + pos * 128 + lane_e
            flat_idx[slot] = ki_h[g_order]
        for k in range(NB):
            ni = int(call_ni[k])
            sb = int(call_slotbase[k])
            a = flat_idx[sb:sb + ni].reshape(-1, 16).T.astype(np.int16)
            idx16[c, :, sb // 16:(sb + ni) // 16] = np.tile(a, (8, 1))

    p = Prep()
    p.chA = tuple(int(x) for x in chA)
    p.chB = tuple(int(x) for x in chB)
    p.idx16 = idx16
    p.dstloc = dstloc
    p.perm = perm
    p.uniqs = uniqs
    return p


def _build_nc(chA, chB):
    from contextlib import ExitStack
    import concourse.tile as tile
    from concourse import bacc, mybir

    fp32 = mybir.dt.float32
    bf16 = mybir.dt.bfloat16
    i32 = mybir.dt.int32
    i16 = mybir.dt.int16
    Alu = mybir.AluOpType
    Act = mybir.ActivationFunctionType

    nc = bacc.Bacc("TRN2", target_bir_lowering=False, debug=False, num_swdge_queues=4)
    nb = NB
    chH = [list(chA), list(chB)]
    cc = [chA[b] + chB[b] for b in range(nb)]
    S = int(sum(cc))
    colbase = [0]
    for x in cc[:-1]:
        colbase.append(colbase[-1] + x)
    call_ni = [128 * (chH[h][2 * g] + chH[h][2 * g + 1])
               for g in range(NGRP) for h in (0, 1)]
    call_icb = [0]
    for x in call_ni[:-1]:
        call_icb.append(call_icb[-1] + x // 16)
    ICOLS = call_icb[-1] + call_ni[-1] // 16
    nchmax = [max(chH[h][2 * g] + chH[h][2 * g + 1] for g in range(NGRP))
              for h in (0, 1)]

    xTc = nc.dram_tensor("xTc", [128, U_PAD], bf16, kind="ExternalInput").ap()
    xTs = nc.dram_tensor("xTs", [128, DST_PAD], bf16, kind="ExternalInput").ap()
    Wq = nc.dram_tensor("Wq", [128, 128], bf16, kind="ExternalInput").ap()
    Wk = nc.dram_tensor("Wk", [128, 128], bf16, kind="ExternalInput").ap()
    Wv = nc.dram_tensor("Wv", [128, 128], bf16, kind="ExternalInput").ap()
    Ws = nc.dram_tensor("Ws", [128, 128], bf16, kind="ExternalInput").ap()
    bq1 = nc.dram_tensor("bq1", [1, 128], bf16, kind="ExternalInput").ap()
    bsv1 = nc.dram_tensor("bsv1", [1, 128], bf16, kind="ExternalInput").ap()
    idx16_d = nc.dram_tensor("idx16", [128, ICOLS], i16, kind="ExternalInput").ap()
    dstloc_d = nc.dram_tensor("dstloc", [128, S], bf16, kind="ExternalInput").ap()

    kv_tab = nc.dram_tensor("kv_tab", [U_PAD, 256], bf16, kind="Internal").ap()
    out_d = nc.dram_tensor("out", [DST_PAD, 128], fp32, kind="ExternalOutput").ap()

    with tile.TileContext(nc) as tc, ExitStack() as ctx:
        const_p = ctx.enter_context(tc.tile_pool(name="const", bufs=1))

        w_q = const_p.tile([128, 128], bf16, tag="wq")
        w_k = const_p.tile([128, 128], bf16, tag="wk")
        w_v = const_p.tile([128, 128], bf16, tag="wv")
        w_s = const_p.tile([128, 128], bf16, tag="ws")
        b_q = const_p.tile([1, 128], bf16, tag="bq")
        b_sv = const_p.tile([1, 128], bf16, tag="bsv")
        nc.sync.dma_start(w_q[:], Wq[:])
        nc.sync.dma_start(w_k[:], Wk[:])
        nc.sync.dma_start(w_v[:], Wv[:])
        nc.sync.dma_start(w_s[:], Ws[:])
        nc.sync.dma_start(b_q[:], bq1[:])
        nc.sync.dma_start(b_sv[:], bsv1[:])

        ones1 = const_p.tile([1, 128], bf16, tag="ones1")
        nc.vector.memset(ones1[:], 1.0)
        ones_col = const_p.tile([128, 1], bf16, tag="ones_col")
        nc.vector.memset(ones_col[:], 1.0)
        iota_i = const_p.tile([128, 128], i32, tag="iota_i")
        nc.gpsimd.iota(iota_i[:], pattern=[[1, 128]], base=0, channel_multiplier=0)
        iota_b = const_p.tile([128, 128], bf16, tag="iota_b")
        nc.vector.tensor_copy(iota_b[:], iota_i[:])
        from concourse.masks import make_identity
        ident = const_p.tile([128, 128], bf16, tag="ident")
        make_identity(nc, ident[:])

        skip_sb = const_p.tile([128, nb, 128], fp32, tag="skip")
        q_sb = const_p.tile([128, nb, 128], bf16, tag="qsb")
        idx16_sb = const_p.tile([128, ICOLS], i16, tag="i16")
        dstloc_sb = const_p.tile([128, S], bf16, tag="dl")
        nc.sync.dma_start(idx16_sb[:], idx16_d[:])
        nc.sync.dma_start(dstloc_sb[:], dstloc_d[:])

        # ---------------- phase 1b: q' and skip for the dst slice ----------------
        with tc.tile_pool(name="p2x", bufs=3) as p2x, \
             tc.tile_pool(name="p2ps", bufs=4, space="PSUM") as p2ps:
            for base in range(0, DST_PAD, TWB):
                w = min(TWB, DST_PAD - base)
                xt = p2x.tile([128, w], bf16, tag="xst")
                nc.sync.dma_start(xt[:], xTs[:, base:base + w])
                for j in range(w // 128):
                    lhs = xt[:, j * 128:(j + 1) * 128]
                    blk = base // 128 + j
                    pq = p2ps.tile([128, 128], fp32, tag="ps2")
                    nc.tensor.matmul(out=pq[:], lhsT=lhs, rhs=w_q[:], start=True, stop=False)
                    nc.tensor.matmul(out=pq[:], lhsT=ones1[:], rhs=b_q[:], start=False, stop=True)
                    ps = p2ps.tile([128, 128], fp32, tag="ps2")
                    nc.tensor.matmul(out=ps[:], lhsT=lhs, rhs=w_s[:], start=True, stop=False)
                    nc.tensor.matmul(out=ps[:], lhsT=ones1[:], rhs=b_sv[:], start=False, stop=True)
                    nc.vector.tensor_copy(q_sb[:, blk, :], pq[:])
                    nc.scalar.activation(skip_sb[:, blk, :], ps[:], Act.Copy)

        # ---------------- phase 1a: k|v table for compacted sources ----------------
        kv_stores = []
        with tc.tile_pool(name="p1x", bufs=3) as p1x, \
             tc.tile_pool(name="p1o", bufs=2) as p1o, \
             tc.tile_pool(name="p1ps", bufs=6, space="PSUM") as p1ps:
            for base in range(0, U_PAD, TW):
                w = min(TW, U_PAD - base)
                nj = w // 128
                xt = p1x.tile([128, w], bf16, tag="xt")
                nc.sync.dma_start(xt[:], xTc[:, base:base + w])
                kvsb = p1o.tile([128, nj, 256], bf16, tag="kvsb")
                for j0 in range(0, nj, 4):
                    js = list(range(j0, min(j0 + 4, nj)))
                    g = len(js)
                    pk = p1ps.tile([128, g * 128], fp32, tag="ps")
                    pv = p1ps.tile([128, g * 128], fp32, tag="ps")
                    for i, j in enumerate(js):
                        lhs = xt[:, j * 128:(j + 1) * 128]
                        nc.tensor.matmul(out=pk[:, i * 128:(i + 1) * 128],
                                         lhsT=lhs, rhs=w_k[:], start=True, stop=True)
                        nc.tensor.matmul(out=pv[:, i * 128:(i + 1) * 128],
                                         lhsT=lhs, rhs=w_v[:], start=True, stop=True)
                    kv = kvsb[:, j0:j0 + g, :]
                    nc.vector.tensor_copy(kv[:, :, 0:128],
                                          pk[:].rearrange("p (c e) -> p c e", e=128))
                    nc.scalar.activation(kv[:, :, 128:256],
                                         pv[:].rearrange("p (c e) -> p c e", e=128),
                                         Act.Copy)
                out_view = kv_tab[base:base + w, :].rearrange("(j p) e -> p j e", p=128)
                kv_stores.append(nc.sync.dma_start(out_view, kvsb[:]))

        # ---------------- phase 2: edge attention + scatter ----------------
        from concourse.tile_rust import add_dep_helper
        first_gather = [None]
        xg_tiles = {}

        with tc.tile_pool(name="gkv", bufs=5) as gkv_p, \
             tc.tile_pool(name="ohp", bufs=3) as oh_p, \
             tc.tile_pool(name="ew", bufs=10) as ew_p, \
             tc.tile_pool(name="mxp", bufs=6) as mx_p, \
             tc.tile_pool(name="epi", bufs=2) as epi_p, \
             tc.tile_pool(name="eps", bufs=2, space="PSUM") as eps_p, \
             tc.tile_pool(name="dps", bufs=2, space="PSUM") as dps_p, \
             tc.tile_pool(name="ops", bufs=2, space="PSUM") as ops_p, \
             tc.tile_pool(name="qps", bufs=2, space="PSUM") as qps_p:

            def emit_gather(g):
                for h in (0, 1):
                    k = g * 2 + h
                    nch = chH[h][2 * g] + chH[h][2 * g + 1]
                    ni = call_ni[k]
                    icb = call_icb[k]
                    t = gkv_p.tile([128, nchmax[h], 256], bf16, tag=f"kv{h}")
                    src = kv_tab[:, :] if h == 0 else kv_tab[HALF:, :]
                    gi = nc.gpsimd.dma_gather(
                        out_ap=t[:, 0:nch, :], in_ap=src,
                        idxs_ap=idx16_sb[:, icb:icb + ni // 16],
                        num_idxs=ni, num_idxs_reg=ni, elem_size=256,
                        transpose=False, single_packet=False,
                        queue_num=k % 4)
                    if first_gather[0] is None:
                        first_gather[0] = gi
                        for s in kv_stores:
                            add_dep_helper(gi.ins, s.ins, reason="kv_tab raw")
                    xg_tiles[(g, h)] = t

            def chunk_slot(b, c):
                h = 0 if c < chA[b] else 1
                lc = c - (chA[b] if h else 0)
                pos = lc + (chH[h][(b // 2) * 2] if b % 2 else 0)
                return xg_tiles[(b // 2, h)], pos

            def pass_a(b):
                cmax = cc[b]
                cb = colbase[b]
                oh_blk = oh_p.tile([128, cmax, 128], bf16, tag="ohb")
                nc.vector.tensor_tensor(
                    out=oh_blk[:],
                    in0=iota_b[:].unsqueeze(1).broadcast_to([128, cmax, 128]),
                    in1=dstloc_sb[:, cb:cb + cmax].unsqueeze(2)
                        .broadcast_to([128, cmax, 128]),
                    op=Alu.is_equal)
                logit_blk = ew_p.tile([128, cmax], fp32, tag="lb")
                for h in (0, 1):
                    nch_h = chH[h][b]
                    lc0 = 0
                    while lc0 < nch_h:
                        w = min(4, nch_h - lc0)
                        cbase = (chA[b] if h else 0) + lc0
                        tile_t, pos = chunk_slot(b, cbase)
                        pot = ops_p.tile([128, w * 128], bf16, tag="pot")
                        for i in range(w):
                            nc.tensor.transpose(out=pot[:, i * 128:(i + 1) * 128],
                                                in_=oh_blk[:, cbase + i, :],
                                                identity=ident[:])
                        ot = ew_p.tile([128, w * 128], bf16, tag="ot")
                        nc.scalar.activation(ot[:], pot[:], Act.Copy)
                        pqg = qps_p.tile([128, w * 128], fp32, tag="pqg")
                        for i in range(w):
                            nc.tensor.matmul(out=pqg[:, i * 128:(i + 1) * 128],
                                             lhsT=ot[:, i * 128:(i + 1) * 128],
                                             rhs=q_sb[:, b, :], start=True, stop=True)
                        prod = ew_p.tile([128, w * 128], bf16, tag="prod")
                        kview = tile_t[:, pos:pos + w, 0:128]
                        nc.vector.tensor_tensor(
                            out=prod[:].rearrange("p (c e) -> p c e", e=128),
                            in0=pqg[:].rearrange("p (c e) -> p c e", e=128),
                            in1=kview, op=Alu.mult)
                        nc.vector.reduce_sum(
                            out=logit_blk[:, cbase:cbase + w],
                            in_=prod[:].rearrange("p (c e) -> p c e", e=128),
                            axis=mybir.AxisListType.X)
                        lc0 += 4
                ex_blk = ew_p.tile([128, cmax], fp32, tag="exb")
                nc.scalar.activation(ex_blk[:], logit_blk[:], Act.Exp)
                return oh_blk, ex_blk

            def pass_b(b, oh_blk, ex_blk):
                cmax = cc[b]
                pagg = eps_p.tile([128, 128], fp32, tag="pagg")
                pden = dps_p.tile([128, 1], fp32, tag="pden")
                for c in range(cmax):
                    mex = mx_p.tile([128, 128], bf16, tag="mex")
                    nc.scalar.activation(mex[:], oh_blk[:, c, :], Act.Copy,
                                         scale=ex_blk[:, c:c + 1])
                    tile_t, pos = chunk_slot(b, c)
                    vslc = tile_t[:, pos, 128:256]
                    nc.tensor.matmul(out=pagg[:], lhsT=mex[:], rhs=vslc,
                                     start=(c == 0), stop=(c == cmax - 1))
                    nc.tensor.matmul(out=pden[:], lhsT=mex[:], rhs=ones_col[:],
                                     start=(c == 0), stop=(c == cmax - 1))
                den = epi_p.tile([128, 1], fp32, tag="den")
                nc.vector.tensor_scalar_add(den[:], pden[:], 1e-30)
                rec = epi_p.tile([128, 1], fp32, tag="rec")
                nc.vector.reciprocal(rec[:], den[:])
                z = epi_p.tile([128, 128], fp32, tag="z")
                nc.scalar.activation(z[:], pagg[:], Act.Copy, scale=rec[:])
                z2 = epi_p.tile([128, 128], fp32, tag="z2")
                nc.vector.tensor_tensor(out=z2[:], in0=z[:], in1=skip_sb[:, b, :], op=Alu.add)
                zn = epi_p.tile([128, 128], fp32, tag="zn")
                nc.vector.tensor_scalar_min(zn[:], z2[:], 0.0)
                en = epi_p.tile([128, 128], fp32, tag="en")
                nc.scalar.activation(en[:], zn[:], Act.Exp)
                zp = epi_p.tile([128, 128], fp32, tag="zp")
                nc.scalar.activation(zp[:], z2[:], Act.Relu)
                o1 = epi_p.tile([128, 128], fp32, tag="o1")
                nc.vector.tensor_tensor(out=o1[:], in0=en[:], in1=zp[:], op=Alu.add)
                o2 = epi_p.tile([128, 128], fp32, tag="o2")
                nc.vector.tensor_scalar_add(o2[:], o1[:], -1.0)
                nc.sync.dma_start(out_d[b * 128:(b + 1) * 128, :], o2[:])

            PREF = 2
            for g in range(min(PREF + 1, NGRP)):
                emit_gather(g)
            prev = None
            for b in range(nb):
                if b % 2 == 0:
                    gg = b // 2 + PREF + 1
                    if gg < NGRP:
                        emit_gather(gg)
                cur = pass_a(b)
                if prev is not None:
                    pass_b(b - 1, *prev)
                prev = cur
            pass_b(nb - 1, *prev)

    nc.compile()
    return nc


_NC_CACHE = {}


def _get_nc(chA, chB):
    key = (chA, chB)
    if key not in _NC_CACHE:
        _NC_CACHE[key] = _build_nc(chA, chB)
    return _NC_CACHE[key]


def _make_in_maps(inputs, prep):
    x = np.asarray(inputs["x"], np.float32)
    xb = x.astype(BF16)
    wq = (np.asarray(inputs["Wq"], np.float32) * SCALE).astype(BF16)
    wk = np.asarray(inputs["Wk"], np.float32).astype(BF16)
    wv = np.asarray(inputs["Wv"], np.float32).astype(BF16)
    ws = np.asarray(inputs["Ws"], np.float32).astype(BF16)
    bq1 = (np.asarray(inputs["bq"], np.float32) * SCALE).astype(BF16).reshape(1, 128)
    bsv1 = (np.asarray(inputs["bs"], np.float32)
            + np.asarray(inputs["bv"], np.float32)).astype(BF16).reshape(1, 128)

    in_maps = []
    for c in range(M_CORES):
        xTc = np.zeros((128, U_PAD), BF16)
        u = prep.uniqs[c]
        xTc[:, :len(u)] = xb[u].T
        xs_local = np.zeros((DST_PAD, 128), BF16)
        xs_local[:DPC] = xb[c * DPC:(c + 1) * DPC]
        xTs = xs_local[np.minimum(prep.perm[c], DST_PAD - 1)].T.copy()
        in_maps.append({
            "xTc": xTc, "xTs": xTs,
            "Wq": wq, "Wk": wk, "Wv": wv, "Ws": ws,
            "bq1": bq1, "bsv1": bsv1,
            "idx16": prep.idx16[c], "dstloc": prep.dstloc[c].astype(BF16),
        })
    return in_maps


def kernel(x, edge_index, Wq, bq, Wk, bk, Wv, bv, Ws, bs):
    from concourse import bass_utils

    prep = _host_prep(edge_index)
    in_maps = _make_in_maps(
        {"x": x, "Wq": Wq, "Wk": Wk, "Wv": Wv, "Ws": Ws,
         "bq": bq, "bs": bs, "bv": bv}, prep)
    nc = _get_nc(prep.chA, prep.chB)
    res = bass_utils.run_bass_kernel_spmd(nc, in_maps, core_ids=list(range(M_CORES)))
    out = np.zeros((N, 128), np.float32)
    for c in range(M_CORES):
        rows = res.results[c]["out"]
        p = prep.perm[c]
        valid = p < DPC
        out[c * DPC + p[valid]] = rows[valid]
    return out
